# revision 2
# baseline (speedup 1.0000x reference)
"""Time-varying FIR (AllZeroDigitalFilter) on 8 TRN2 NeuronCores — v3.1.

Structure (per core: 2 sequences x 8 chunks x 126 frame-rows):
  C[k, i'] = sum_j h[k, j] * x[k*80 + i' - j + pad],  i' in [0,160)
  y[k*80+i] = w1[i]*C_{k+1}[i] + w0[i]*C_k[80+i]

Division of labor, all ops wide (FD = 8 chunks * 160 = 1280) to
amortize fixed per-instruction costs:
  DVE: 37 taps/seq as one wide mult each (in1 = per-(frame,chunk) h
       broadcast via stride-0 inner AP; 1x mode) -> product planes
  ACT: 13 taps/seq as 8 chunk-wise Copy-with-scale products -> planes
  PE : accumulates every plane into PSUM via identity-stationary
       matmuls (3 windows 480|480|320; fp32 accumulation; ldw-opt
       dedups the identity LDWEIGHTS); also applies the +1 partition
       shift with a constant superdiagonal matrix (Vs = SH.T @ V),
       replacing a slow strided SBUF->SBUF DMA
  DVE tail: V = psum*rr; y = V_hi + Vs_psum (fp32 out)
GpSimd idle by design: its tensor ops halve concurrent DVE throughput
(shared SBUF port).

Hard-won rules encoded here:
  - PE must never write PSUM (any bank) while DVE reads PSUM: PE is
    gated on v_s / y_s around the V and blend phases.
  - Completion semaphores ride on the last real engine instruction,
    never on a trailing nop (fires before queued PSUM writes land).
  - Per-DMA semaphores (completion skew across SDMA engines makes
    shared counters unsound).
  - Input is host-prepped into the unfolded [126, chunk, 212] layout
    (contiguous DMA; the strided gather cost ~12us of head latency),
    output staged [125, 1280] and de-interleaved on host.

Sharding: pure data parallel across batch, 2 sequences per core.
"""

import sys

for p in ("/opt/trn_rl_repo", "/root/.axon_site/_ro/trn_rl_repo"):
    if p not in sys.path:
        sys.path.append(p)

import numpy as np
import concourse.bass as bass
import concourse.mybir as mybir
from concourse.ap import AP
from concourse.bass_utils import run_bass_kernel_spmd


def _enable_ldw_opt():
    """Dedup identical LDWEIGHTS (walrus --enable-ldw-opt): saves ~40us of
    redundant identity reloads on the PE. Idempotent monkeypatch."""
    from concourse import bass_utils as _bu

    if getattr(_bu.run_command, "_ldw_opt_patched", False):
        return
    _orig = _bu.run_command

    def _patched(cmd, **kw):
        cmd = [
            "--enable-ldw-opt=true" if c == "--enable-ldw-opt=false" else c
            for c in cmd
        ]
        return _orig(cmd, **kw)

    _patched._ldw_opt_patched = True
    _bu.run_command = _patched


_enable_ldw_opt()

B, T = 16, 80000
P, D = 80, 50
N = T // P
NCORES = 8
S = B // NCORES
FO = 125
NC = 8
CS = 212
PAD = D - 1 + P  # 129
GF = NC * 160  # 1280
XF = NC * CS  # 1696
WIN = [(0, 480), (480, 480), (960, 320)]

F16 = mybir.dt.float16
FP32 = mybir.dt.float32

DVE_TAPS = list(range(0, 37))
GPS_TAPS = []
ACT_TAPS = list(range(37, 50))

_nc_cache = {}


def plane_schedule():
    """PE consume order. D-planes arrive every ~1.5us, A-planes every
    ~4.04us; A-ready is biased +2.5us so the fast consumer doesn't
    head-of-line block on the slow producer while D-planes queue."""
    ev = [("D", k, 1500.0 * (k + 1)) for k in range(len(DVE_TAPS))]
    ev += [("A", k, 4040.0 * (k + 1) + 4500.0) for k in range(len(ACT_TAPS))]
    ev.sort(key=lambda e: e[2])
    return [(kind, k) for kind, k, _ in ev]


def build_nc():
    if "nc" in _nc_cache:
        return _nc_cache["nc"]
    nc = bass.Bass()
    xa_ext = nc.declare_dram_parameter("xae", [S, 126, XF], F16, isOutput=False)
    hg_ext = nc.declare_dram_parameter("hg", [126, D * 16], F16, isOutput=False)
    hg32_ext = nc.declare_dram_parameter("hg32", [126, D * 16], FP32, isOutput=False)
    rr_ext = nc.declare_dram_parameter("rr", [128, 160], F16, isOutput=False)
    eye_ext = nc.declare_dram_parameter("eye", [126, 126], F16, isOutput=False)
    shm_ext = nc.declare_dram_parameter("shm", [126, 126], F16, isOutput=False)
    out_ext = nc.declare_dram_parameter("out", [S, FO, GF // 2], FP32, isOutput=True)

    from contextlib import ExitStack

    with ExitStack() as _ctx:
        ec = _ctx.enter_context
        xa0 = ec(nc.sbuf_tensor([126, XF], F16))
        xa1 = ec(nc.sbuf_tensor([126, XF], F16))
        hgx = ec(nc.sbuf_tensor([126, D * 16], F16))
        hgx32 = ec(nc.sbuf_tensor([126, D * 16], FP32))
        rrt = ec(nc.sbuf_tensor([128, 160], F16))
        eye = ec(nc.sbuf_tensor([126, 126], F16))
        shm = ec(nc.sbuf_tensor([126, 126], F16))
        dpb = [ec(nc.sbuf_tensor(f"dpl{i}", [126, GF], F16)) for i in range(12)]
        gpb = [ec(nc.sbuf_tensor(f"gpl{i}", [126, GF], F16)) for i in range(2)]
        apb = [ec(nc.sbuf_tensor(f"apl{i}", [126, GF], F16)) for i in range(4)]
        vt0 = ec(nc.sbuf_tensor([126, GF], F16))
        vt1 = ec(nc.sbuf_tensor([126, GF], F16))
        yy0 = ec(nc.sbuf_tensor([125, GF // 2], FP32))
        yy1 = ec(nc.sbuf_tensor([125, GF // 2], FP32))
        ps0_0 = ec(nc.psum_tensor("ps0_0", [126, 480], FP32))
        ps0_1 = ec(nc.psum_tensor("ps0_1", [126, 480], FP32))
        ps0_2 = ec(nc.psum_tensor("ps0_2", [126, 320], FP32))
        ps1_0 = ec(nc.psum_tensor("ps1_0", [126, 480], FP32))
        ps1_1 = ec(nc.psum_tensor("ps1_1", [126, 480], FP32))
        ps1_2 = ec(nc.psum_tensor("ps1_2", [126, 320], FP32))
        xin0 = ec(nc.semaphore("xin0"))
        xin0b = ec(nc.semaphore("xin0b"))
        xin1 = ec(nc.semaphore("xin1"))
        xin1b = ec(nc.semaphore("xin1b"))
        o_s2 = ec(nc.semaphore("o_s2"))
        cin = ec(nc.semaphore("cin"))
        hgin = ec(nc.semaphore("hgin"))
        pein = ec(nc.semaphore("pein"))
        dp_s = ec(nc.semaphore("dp_s"))
        dc_s = ec(nc.semaphore("dc_s"))
        ap_s = ec(nc.semaphore("ap_s"))
        ac_s = ec(nc.semaphore("ac_s"))
        gp_s = ec(nc.semaphore("gp_s"))
        gc_s = ec(nc.semaphore("gc_s"))
        pe_d = ec(nc.semaphore("pe_d"))
        v_s = ec(nc.semaphore("v_s"))
        vsr = ec(nc.semaphore("vsr"))
        y_s = ec(nc.semaphore("y_s"))
        o_s = ec(nc.semaphore("o_s"))
        block = ec(nc.Block())

        xa = [xa0, xa1]
        xin = [xin0, xin1]
        vt = [vt0, vt1]
        yy = [yy0, yy1]
        psall = [[ps0_0, ps0_1, ps0_2], [ps1_0, ps1_1, ps1_2]]

        def xa3(s, j):
            return AP(
                tensor=xa[s][:].tensor,
                offset=(D - 1 - j),
                ap=[[XF, 126], [CS, NC], [1, 160]],
            )

        def xa2(s, j, c):
            return xa[s][:, c * CS + D - 1 - j : c * CS + D - 1 - j + 160]

        def hb(s, j):
            return AP(
                tensor=hgx[:].tensor,
                offset=j * 16 + s * NC,
                ap=[[D * 16, 126], [1, NC], [0, 160]],
            )

        def rrb(nw):
            return AP(
                tensor=rrt[:].tensor, offset=0, ap=[[160, 126], [0, nw], [1, 160]]
            )

        @block.sync
        def _(sync):
            sync.dma_start(xa[0][0:63, :], xa_ext[0, 0:63, :]).then_inc(xin[0], 16)
            sync.dma_start(hgx[:], hg_ext[:]).then_inc(hgin, 16)
            sync.dma_start(rrt[:], rr_ext[:]).then_inc(hgin, 16)
            sync.dma_start(hgx32[:], hg32_ext[:]).then_inc(cin, 16)
            sync.dma_start(eye[:], eye_ext[:]).then_inc(pein, 16)
            sync.dma_start(shm[:], shm_ext[:]).then_inc(pein, 16)
            sync.dma_start(xa[1][0:63, :], xa_ext[1, 0:63, :]).then_inc(xin[1], 16)
            sync.dma_start(xa[1][63:126, :], xa_ext[1, 63:126, :]).then_inc(
                xin1b, 16
            )
            sync.wait_ge(y_s, 1)
            sync.dma_start(out_ext[0, :, :], yy[0][:]).then_inc(o_s, 16)
            sync.wait_ge(y_s, 2)
            sync.dma_start(out_ext[1, :, 0:320], yy[1][:, 0:320]).then_inc(o_s, 16)
            sync.wait_ge(o_s, 32)
            sync.wait_ge(o_s2, 16)

        def emit_vmult(vector, s):
            vector.wait_ge(pe_d, s + 1)
            for w, (o, ln) in enumerate(WIN):
                inst = vector.tensor_tensor(
                    out=vt[s][:, o : o + ln],
                    in0=psall[s][w][:],
                    in1=rrb(ln // 160),
                    op=mybir.AluOpType.mult,
                )
            inst.then_inc(v_s, 1)

        def emit_yblend(vector, s):
            # y[q, c, i] = V[q, c, 80+i] + Vs[q, c, i]; Vs lives in psum
            vector.wait_ge(vsr, s + 1)
            for w, (o, ln) in enumerate(WIN):
                nw = ln // 160
                inst = vector.tensor_tensor(
                    out=AP(
                        tensor=yy[s][:].tensor,
                        offset=o // 2,
                        ap=[[GF // 2, 125], [P, nw], [1, P]],
                    ),
                    in0=AP(
                        tensor=vt[s][:].tensor,
                        offset=o + P,
                        ap=[[GF, 125], [160, nw], [1, P]],
                    ),
                    in1=AP(
                        tensor=psall[s][w][:].tensor,
                        offset=0,
                        ap=[[ln, 125], [160, nw], [1, P]],
                    ),
                    op=mybir.AluOpType.add,
                )
            inst.then_inc(y_s, 1)

        @block.vector
        def _(vector):
            vector.wait_ge(hgin, 32)
            nd = len(DVE_TAPS)
            xinb = [xin0b, xin1b]
            for s in range(S):
                vector.wait_ge(xin[s], 16)
                vector.wait_ge(xinb[s], 16)
                for k, j in enumerate(DVE_TAPS):
                    g = s * nd + k + 1
                    if g > 12:
                        vector.wait_ge(dc_s, (g - 12 + 3) // 4)
                    vector.tensor_tensor(
                        out=dpb[(g - 1) % 12][:],
                        in0=xa3(s, j),
                        in1=hb(s, j),
                        op=mybir.AluOpType.mult,
                    ).then_inc(dp_s, 1)
                    if s == 1 and k == 1:
                        emit_vmult(vector, 0)
                    if s == 1 and k == 5:
                        emit_yblend(vector, 0)
            emit_vmult(vector, 1)
            emit_yblend(vector, 1)

        @block.tensor
        def _(tensor):
            tensor.wait_ge(pein, 32)
            nd, na = len(DVE_TAPS), len(ACT_TAPS)
            sched = plane_schedule()
            nplanes = nd + na + len(GPS_TAPS)
            for s in range(S):
                if s > 0:
                    # PE may not touch PSUM while DVE reads it (V/blend of
                    # the previous sequence)
                    tensor.wait_ge(y_s, s)
                done = 0
                for kind, k in sched:
                    if kind == "D":
                        g = s * nd + k + 1
                        tensor.wait_ge(dp_s, g)
                        src = dpb[(g - 1) % 12]
                    elif kind == "G":
                        g = s * len(GPS_TAPS) + k + 1
                        tensor.wait_ge(gp_s, g)
                        src = gpb[(g - 1) % 2]
                    else:
                        g = s * na + k + 1
                        tensor.wait_ge(ap_s, g)
                        src = apb[(g - 1) % 4]
                    first = done == 0
                    done += 1
                    last = done == nplanes
                    for w, (o, ln) in enumerate(WIN):
                        inst = tensor.matmul(
                            psall[s][w][:],
                            eye[:],
                            src[:, o : o + ln],
                            start=first,
                            stop=last,
                            skip_group_check=True,
                        )
                    if done == nplanes:
                        inst.then_inc(pe_d, 1)
                    elif kind == "D" and g % 4 == 0:
                        inst.then_inc(dc_s, 1)
                    elif kind == "G":
                        inst.then_inc(gc_s, 1)
                    elif kind == "A" and g % 2 == 0:
                        inst.then_inc(ac_s, 1)
                # Vs = SH.T @ V (the +1 partition shift), reusing this
                # sequence's psum banks after the V-mult consumed them
                tensor.wait_ge(v_s, s + 1)
                for w, (o, ln) in enumerate(WIN):
                    inst = tensor.matmul(
                        psall[s][w][:],
                        shm[:],
                        vt[s][:, o : o + ln],
                        start=True,
                        stop=True,
                        skip_group_check=True,
                    )
                inst.then_inc(vsr, 1)

        @block.scalar
        def _(scalar):
            scalar.dma_start(xa[0][63:126, :], xa_ext[0, 63:126, :]).then_inc(
                xin0b, 16
            )
            scalar.wait_ge(cin, 16)
            xinb = [xin0b, xin1b]
            na = len(ACT_TAPS)
            for s in range(S):
                scalar.wait_ge(xinb[s], 16)
                scalar.wait_ge(xin[s], 16)
                for k, j in enumerate(ACT_TAPS):
                    g = s * na + k + 1
                    if g > 4:
                        scalar.wait_ge(ac_s, (g - 4 + 1) // 2)
                    for c in range(NC):
                        inst = scalar.activation(
                            apb[(g - 1) % 4][:, c * 160 : (c + 1) * 160],
                            xa2(s, j, c),
                            mybir.ActivationFunctionType.Copy,
                            scale=hgx32[
                                :, j * 16 + s * NC + c : j * 16 + s * NC + c + 1
                            ],
                        )
                    inst.then_inc(ap_s, 1)
            scalar.wait_ge(y_s, 2)
            scalar.dma_start(out_ext[1, :, 320:640], yy[1][:, 320:640]).then_inc(
                o_s2, 16
            )

    _nc_cache["nc"] = nc
    return nc


def _prep_core_inputs(x, h):
    x = np.ascontiguousarray(x, dtype=np.float32)
    h = np.ascontiguousarray(h, dtype=np.float32)
    TPX = N * P + CS + 4
    xp = np.zeros((B, TPX), np.float16)
    xp[:, PAD : PAD + T] = x.astype(np.float16)
    # host im2col: xae[b, q, c*CS + u] = xp[b, (c*FO + q)*P + u]
    idx_q = (np.arange(126)[:, None, None] * P + np.arange(NC)[None, :, None] * FO * P
             + np.arange(CS)[None, None, :])  # [126, NC, CS]
    xae_all = xp[:, idx_q.reshape(-1)].reshape(B, 126, NC * CS)
    hpad = np.concatenate([h, h[:, -1:, :]], axis=1)
    w1 = (np.arange(P, dtype=np.float32) / P).astype(np.float16)
    w0 = (1.0 - np.arange(P, dtype=np.float32) / P).astype(np.float16)
    rr = np.ascontiguousarray(
        np.broadcast_to(np.concatenate([w1, w0])[None, :], (128, 160))
    )
    eye = np.eye(126, dtype=np.float16)
    shm = np.zeros((126, 126), np.float16)
    shm[np.arange(1, 126), np.arange(125)] = 1.0  # SH[k', m] = 1 iff k' == m+1
    in_maps = []
    for core in range(NCORES):
        sl = slice(core * S, (core + 1) * S)
        hc = hpad[sl]
        hg32 = np.zeros((126, D * 16), np.float32)
        for s in range(S):
            for c in range(NC):
                blk = hc[s, c * FO : c * FO + 126, :]
                hg32[:, s * NC + c :: 16] = blk
        in_maps.append(
            {
                "xae": np.ascontiguousarray(xae_all[sl]),
                "hg": hg32.astype(np.float16),
                "hg32": hg32,
                "rr": rr,
                "eye": eye,
                "shm": shm,
            }
        )
    return in_maps


def _unstage(res):
    outs = []
    for c in range(NCORES):
        o = res.results[c]["out"]  # [S, 125, 8*80]
        o = o.reshape(S, FO, NC, P).transpose(0, 2, 1, 3).reshape(S, T)
        outs.append(o)
    return np.ascontiguousarray(np.concatenate(outs, axis=0), dtype=np.float32)


def kernel(x, h, **kw):
    nc = build_nc()
    in_maps = _prep_core_inputs(x, h)
    res = run_bass_kernel_spmd(nc, in_maps, core_ids=list(range(NCORES)), **kw)
    return _unstage(res)


def kernel_traced(x, h, **kw):
    nc = build_nc()
    in_maps = _prep_core_inputs(x, h)
    res = run_bass_kernel_spmd(
        nc, in_maps, core_ids=list(range(NCORES)), trace=True, **kw
    )
    return _unstage(res), res


# revision 3
# speedup vs baseline: 1.0166x; 1.0166x over previous
"""Time-varying FIR (AllZeroDigitalFilter) on 8 TRN2 NeuronCores — v3.1.

Structure (per core: 2 sequences x 8 chunks x 126 frame-rows):
  C[k, i'] = sum_j h[k, j] * x[k*80 + i' - j + pad],  i' in [0,160)
  y[k*80+i] = w1[i]*C_{k+1}[i] + w0[i]*C_k[80+i]

Division of labor, all ops wide (FD = 8 chunks * 160 = 1280) to
amortize fixed per-instruction costs:
  DVE: 37 taps/seq as one wide mult each (in1 = per-(frame,chunk) h
       broadcast via stride-0 inner AP; 1x mode) -> product planes
  ACT: 13 taps/seq as 8 chunk-wise Copy-with-scale products -> planes
  PE : accumulates every plane into PSUM via identity-stationary
       matmuls (3 windows 480|480|320; fp32 accumulation; ldw-opt
       dedups the identity LDWEIGHTS); also applies the +1 partition
       shift with a constant superdiagonal matrix (Vs = SH.T @ V),
       replacing a slow strided SBUF->SBUF DMA
  DVE tail: V = psum*rr; y = V_hi + Vs_psum (fp32 out)
GpSimd idle by design: its tensor ops halve concurrent DVE throughput
(shared SBUF port).

Hard-won rules encoded here:
  - PE must never write PSUM (any bank) while DVE reads PSUM: PE is
    gated on v_s / y_s around the V and blend phases.
  - Completion semaphores ride on the last real engine instruction,
    never on a trailing nop (fires before queued PSUM writes land).
  - Per-DMA semaphores (completion skew across SDMA engines makes
    shared counters unsound).
  - Input is host-prepped into the unfolded [126, chunk, 212] layout
    (contiguous DMA; the strided gather cost ~12us of head latency),
    output staged [125, 1280] and de-interleaved on host.

Sharding: pure data parallel across batch, 2 sequences per core.
"""

import sys

for p in ("/opt/trn_rl_repo", "/root/.axon_site/_ro/trn_rl_repo"):
    if p not in sys.path:
        sys.path.append(p)

import numpy as np
import concourse.bass as bass
import concourse.mybir as mybir
from concourse.ap import AP
from concourse.bass_utils import run_bass_kernel_spmd


def _enable_ldw_opt():
    """Dedup identical LDWEIGHTS (walrus --enable-ldw-opt): saves ~40us of
    redundant identity reloads on the PE. Idempotent monkeypatch."""
    from concourse import bass_utils as _bu

    if getattr(_bu.run_command, "_ldw_opt_patched", False):
        return
    _orig = _bu.run_command

    def _patched(cmd, **kw):
        cmd = [
            "--enable-ldw-opt=true" if c == "--enable-ldw-opt=false" else c
            for c in cmd
        ]
        return _orig(cmd, **kw)

    _patched._ldw_opt_patched = True
    _bu.run_command = _patched


_enable_ldw_opt()

B, T = 16, 80000
P, D = 80, 50
N = T // P
NCORES = 8
S = B // NCORES
FO = 125
NC = 8
CS = 212
PAD = D - 1 + P  # 129
GF = NC * 160  # 1280
XF = NC * CS  # 1696
WIN = [(0, 480), (480, 480), (960, 320)]

F16 = mybir.dt.float16
FP32 = mybir.dt.float32

DVE_TAPS = list(range(0, 37))
GPS_TAPS = []
ACT_TAPS = list(range(37, 50))

_nc_cache = {}


def plane_schedule():
    """PE consume order. D-planes arrive every ~1.5us, A-planes every
    ~4.04us; A-ready is biased +2.5us so the fast consumer doesn't
    head-of-line block on the slow producer while D-planes queue."""
    ev = [("D", k, 1500.0 * (k + 1)) for k in range(len(DVE_TAPS))]
    ev += [("A", k, 4040.0 * (k + 1) + 6500.0) for k in range(len(ACT_TAPS))]
    ev.sort(key=lambda e: e[2])
    return [(kind, k) for kind, k, _ in ev]


def build_nc():
    if "nc" in _nc_cache:
        return _nc_cache["nc"]
    nc = bass.Bass()
    xa_ext = nc.declare_dram_parameter("xae", [S, 126, XF], F16, isOutput=False)
    hg_ext = nc.declare_dram_parameter("hg", [126, D * 16], F16, isOutput=False)
    hg32_ext = nc.declare_dram_parameter("hg32", [126, D * 16], FP32, isOutput=False)
    rr_ext = nc.declare_dram_parameter("rr", [128, 160], F16, isOutput=False)
    eye_ext = nc.declare_dram_parameter("eye", [126, 126], F16, isOutput=False)
    shm_ext = nc.declare_dram_parameter("shm", [126, 126], F16, isOutput=False)
    out_ext = nc.declare_dram_parameter("out", [S, FO, GF // 2], FP32, isOutput=True)

    from contextlib import ExitStack

    with ExitStack() as _ctx:
        ec = _ctx.enter_context
        xa0 = ec(nc.sbuf_tensor([126, XF], F16))
        xa1 = ec(nc.sbuf_tensor([126, XF], F16))
        hgx = ec(nc.sbuf_tensor([126, D * 16], F16))
        hgx32 = ec(nc.sbuf_tensor([126, D * 16], FP32))
        rrt = ec(nc.sbuf_tensor([128, 160], F16))
        eye = ec(nc.sbuf_tensor([126, 126], F16))
        shm = ec(nc.sbuf_tensor([126, 126], F16))
        dpb = [ec(nc.sbuf_tensor(f"dpl{i}", [126, GF], F16)) for i in range(12)]
        gpb = [ec(nc.sbuf_tensor(f"gpl{i}", [126, GF], F16)) for i in range(2)]
        apb = [ec(nc.sbuf_tensor(f"apl{i}", [126, GF], F16)) for i in range(6)]
        vt0 = ec(nc.sbuf_tensor([126, GF], F16))
        vt1 = ec(nc.sbuf_tensor([126, GF], F16))
        yy0 = ec(nc.sbuf_tensor([125, GF // 2], FP32))
        yy1 = ec(nc.sbuf_tensor([125, GF // 2], FP32))
        ps0_0 = ec(nc.psum_tensor("ps0_0", [126, 480], FP32))
        ps0_1 = ec(nc.psum_tensor("ps0_1", [126, 480], FP32))
        ps0_2 = ec(nc.psum_tensor("ps0_2", [126, 320], FP32))
        ps1_0 = ec(nc.psum_tensor("ps1_0", [126, 480], FP32))
        ps1_1 = ec(nc.psum_tensor("ps1_1", [126, 480], FP32))
        ps1_2 = ec(nc.psum_tensor("ps1_2", [126, 320], FP32))
        xin0 = ec(nc.semaphore("xin0"))
        xin0b = ec(nc.semaphore("xin0b"))
        xin1 = ec(nc.semaphore("xin1"))
        xin1b = ec(nc.semaphore("xin1b"))
        o_s2 = ec(nc.semaphore("o_s2"))
        cin = ec(nc.semaphore("cin"))
        hgin = ec(nc.semaphore("hgin"))
        pein = ec(nc.semaphore("pein"))
        dp_s = ec(nc.semaphore("dp_s"))
        dc_s = ec(nc.semaphore("dc_s"))
        ap_s = ec(nc.semaphore("ap_s"))
        ac_s = ec(nc.semaphore("ac_s"))
        gp_s = ec(nc.semaphore("gp_s"))
        gc_s = ec(nc.semaphore("gc_s"))
        pe_d = ec(nc.semaphore("pe_d"))
        v_s = ec(nc.semaphore("v_s"))
        vsr = ec(nc.semaphore("vsr"))
        y_s = ec(nc.semaphore("y_s"))
        o_s = ec(nc.semaphore("o_s"))
        block = ec(nc.Block())

        xa = [xa0, xa1]
        xin = [xin0, xin1]
        vt = [vt0, vt1]
        yy = [yy0, yy1]
        psall = [[ps0_0, ps0_1, ps0_2], [ps1_0, ps1_1, ps1_2]]

        def xa3(s, j):
            return AP(
                tensor=xa[s][:].tensor,
                offset=(D - 1 - j),
                ap=[[XF, 126], [CS, NC], [1, 160]],
            )

        def xa2(s, j, c):
            return xa[s][:, c * CS + D - 1 - j : c * CS + D - 1 - j + 160]

        def hb(s, j):
            return AP(
                tensor=hgx[:].tensor,
                offset=j * 16 + s * NC,
                ap=[[D * 16, 126], [1, NC], [0, 160]],
            )

        def rrb(nw):
            return AP(
                tensor=rrt[:].tensor, offset=0, ap=[[160, 126], [0, nw], [1, 160]]
            )

        @block.sync
        def _(sync):
            sync.dma_start(xa[0][0:63, :], xa_ext[0, 0:63, :]).then_inc(xin[0], 16)
            sync.dma_start(hgx[:], hg_ext[:]).then_inc(hgin, 16)
            sync.dma_start(rrt[:], rr_ext[:]).then_inc(cin, 16)
            sync.dma_start(hgx32[:], hg32_ext[:]).then_inc(cin, 16)
            sync.dma_start(eye[:], eye_ext[:]).then_inc(pein, 16)
            sync.dma_start(shm[:], shm_ext[:]).then_inc(pein, 16)
            sync.dma_start(xa[1][0:63, :], xa_ext[1, 0:63, :]).then_inc(xin[1], 16)
            sync.dma_start(xa[1][63:126, :], xa_ext[1, 63:126, :]).then_inc(
                xin1b, 16
            )
            sync.wait_ge(y_s, 1)
            sync.dma_start(out_ext[0, :, :], yy[0][:]).then_inc(o_s, 16)
            sync.wait_ge(y_s, 2)
            sync.dma_start(out_ext[1, :, 0:320], yy[1][:, 0:320]).then_inc(o_s, 16)
            sync.wait_ge(o_s, 32)
            sync.wait_ge(o_s2, 16)

        def emit_vmult(vector, s):
            if s == 0:
                vector.wait_ge(cin, 32)
            vector.wait_ge(pe_d, s + 1)
            for w, (o, ln) in enumerate(WIN):
                inst = vector.tensor_tensor(
                    out=vt[s][:, o : o + ln],
                    in0=psall[s][w][:],
                    in1=rrb(ln // 160),
                    op=mybir.AluOpType.mult,
                )
            inst.then_inc(v_s, 1)

        def emit_yblend(vector, s):
            # y[q, c, i] = V[q, c, 80+i] + Vs[q, c, i]; Vs lives in psum
            vector.wait_ge(vsr, s + 1)
            for w, (o, ln) in enumerate(WIN):
                nw = ln // 160
                inst = vector.tensor_tensor(
                    out=AP(
                        tensor=yy[s][:].tensor,
                        offset=o // 2,
                        ap=[[GF // 2, 125], [P, nw], [1, P]],
                    ),
                    in0=AP(
                        tensor=vt[s][:].tensor,
                        offset=o + P,
                        ap=[[GF, 125], [160, nw], [1, P]],
                    ),
                    in1=AP(
                        tensor=psall[s][w][:].tensor,
                        offset=0,
                        ap=[[ln, 125], [160, nw], [1, P]],
                    ),
                    op=mybir.AluOpType.add,
                )
            inst.then_inc(y_s, 1)

        @block.vector
        def _(vector):
            vector.wait_ge(hgin, 16)
            nd = len(DVE_TAPS)
            xinb = [xin0b, xin1b]
            for s in range(S):
                vector.wait_ge(xin[s], 16)
                vector.wait_ge(xinb[s], 16)
                for k, j in enumerate(DVE_TAPS):
                    g = s * nd + k + 1
                    if g > 12:
                        vector.wait_ge(dc_s, (g - 12 + 3) // 4)
                    vector.tensor_tensor(
                        out=dpb[(g - 1) % 12][:],
                        in0=xa3(s, j),
                        in1=hb(s, j),
                        op=mybir.AluOpType.mult,
                    ).then_inc(dp_s, 1)
                    if s == 1 and k == 1:
                        emit_vmult(vector, 0)
                    if s == 1 and k == 5:
                        emit_yblend(vector, 0)
            emit_vmult(vector, 1)
            emit_yblend(vector, 1)

        @block.tensor
        def _(tensor):
            tensor.wait_ge(pein, 32)
            nd, na = len(DVE_TAPS), len(ACT_TAPS)
            sched = plane_schedule()
            nplanes = nd + na + len(GPS_TAPS)
            for s in range(S):
                if s > 0:
                    # PE may not touch PSUM while DVE reads it (V/blend of
                    # the previous sequence)
                    tensor.wait_ge(y_s, s)
                done = 0
                for kind, k in sched:
                    if kind == "D":
                        g = s * nd + k + 1
                        tensor.wait_ge(dp_s, g)
                        src = dpb[(g - 1) % 12]
                    elif kind == "G":
                        g = s * len(GPS_TAPS) + k + 1
                        tensor.wait_ge(gp_s, g)
                        src = gpb[(g - 1) % 2]
                    else:
                        g = s * na + k + 1
                        tensor.wait_ge(ap_s, g)
                        src = apb[(g - 1) % 6]
                    first = done == 0
                    done += 1
                    last = done == nplanes
                    for w, (o, ln) in enumerate(WIN):
                        inst = tensor.matmul(
                            psall[s][w][:],
                            eye[:],
                            src[:, o : o + ln],
                            start=first,
                            stop=last,
                            skip_group_check=True,
                        )
                    if done == nplanes:
                        inst.then_inc(pe_d, 1)
                    elif kind == "D" and g % 4 == 0:
                        inst.then_inc(dc_s, 1)
                    elif kind == "G":
                        inst.then_inc(gc_s, 1)
                    elif kind == "A" and g % 2 == 0:
                        inst.then_inc(ac_s, 1)
                # Vs = SH.T @ V (the +1 partition shift), reusing this
                # sequence's psum banks after the V-mult consumed them
                tensor.wait_ge(v_s, s + 1)
                for w, (o, ln) in enumerate(WIN):
                    inst = tensor.matmul(
                        psall[s][w][:],
                        shm[:],
                        vt[s][:, o : o + ln],
                        start=True,
                        stop=True,
                        skip_group_check=True,
                    )
                inst.then_inc(vsr, 1)

        @block.scalar
        def _(scalar):
            scalar.dma_start(xa[0][63:126, :], xa_ext[0, 63:126, :]).then_inc(
                xin0b, 16
            )
            scalar.wait_ge(cin, 32)
            xinb = [xin0b, xin1b]
            na = len(ACT_TAPS)
            for s in range(S):
                scalar.wait_ge(xinb[s], 16)
                scalar.wait_ge(xin[s], 16)
                for k, j in enumerate(ACT_TAPS):
                    g = s * na + k + 1
                    if g > 6:
                        scalar.wait_ge(ac_s, (g - 6 + 1) // 2)
                    for c in range(NC):
                        inst = scalar.activation(
                            apb[(g - 1) % 6][:, c * 160 : (c + 1) * 160],
                            xa2(s, j, c),
                            mybir.ActivationFunctionType.Copy,
                            scale=hgx32[
                                :, j * 16 + s * NC + c : j * 16 + s * NC + c + 1
                            ],
                        )
                    inst.then_inc(ap_s, 1)
            scalar.wait_ge(y_s, 2)
            scalar.dma_start(out_ext[1, :, 320:640], yy[1][:, 320:640]).then_inc(
                o_s2, 16
            )

    _nc_cache["nc"] = nc
    return nc


def _prep_core_inputs(x, h):
    x = np.ascontiguousarray(x, dtype=np.float32)
    h = np.ascontiguousarray(h, dtype=np.float32)
    TPX = N * P + CS + 4
    xp = np.zeros((B, TPX), np.float16)
    xp[:, PAD : PAD + T] = x.astype(np.float16)
    # host im2col: xae[b, q, c*CS + u] = xp[b, (c*FO + q)*P + u]
    idx_q = (np.arange(126)[:, None, None] * P + np.arange(NC)[None, :, None] * FO * P
             + np.arange(CS)[None, None, :])  # [126, NC, CS]
    xae_all = xp[:, idx_q.reshape(-1)].reshape(B, 126, NC * CS)
    hpad = np.concatenate([h, h[:, -1:, :]], axis=1)
    w1 = (np.arange(P, dtype=np.float32) / P).astype(np.float16)
    w0 = (1.0 - np.arange(P, dtype=np.float32) / P).astype(np.float16)
    rr = np.ascontiguousarray(
        np.broadcast_to(np.concatenate([w1, w0])[None, :], (128, 160))
    )
    eye = np.eye(126, dtype=np.float16)
    shm = np.zeros((126, 126), np.float16)
    shm[np.arange(1, 126), np.arange(125)] = 1.0  # SH[k', m] = 1 iff k' == m+1
    in_maps = []
    for core in range(NCORES):
        sl = slice(core * S, (core + 1) * S)
        hc = hpad[sl]
        hg32 = np.zeros((126, D * 16), np.float32)
        for s in range(S):
            for c in range(NC):
                blk = hc[s, c * FO : c * FO + 126, :]
                hg32[:, s * NC + c :: 16] = blk
        in_maps.append(
            {
                "xae": np.ascontiguousarray(xae_all[sl]),
                "hg": hg32.astype(np.float16),
                "hg32": hg32,
                "rr": rr,
                "eye": eye,
                "shm": shm,
            }
        )
    return in_maps


def _unstage(res):
    outs = []
    for c in range(NCORES):
        o = res.results[c]["out"]  # [S, 125, 8*80]
        o = o.reshape(S, FO, NC, P).transpose(0, 2, 1, 3).reshape(S, T)
        outs.append(o)
    return np.ascontiguousarray(np.concatenate(outs, axis=0), dtype=np.float32)


def kernel(x, h, **kw):
    nc = build_nc()
    in_maps = _prep_core_inputs(x, h)
    res = run_bass_kernel_spmd(nc, in_maps, core_ids=list(range(NCORES)), **kw)
    return _unstage(res)


def kernel_traced(x, h, **kw):
    nc = build_nc()
    in_maps = _prep_core_inputs(x, h)
    res = run_bass_kernel_spmd(
        nc, in_maps, core_ids=list(range(NCORES)), trace=True, **kw
    )
    return _unstage(res), res


# revision 4
# speedup vs baseline: 1.0170x; 1.0003x over previous
"""Time-varying FIR (AllZeroDigitalFilter) on 8 TRN2 NeuronCores — v3.1.

Structure (per core: 2 sequences x 8 chunks x 126 frame-rows):
  C[k, i'] = sum_j h[k, j] * x[k*80 + i' - j + pad],  i' in [0,160)
  y[k*80+i] = w1[i]*C_{k+1}[i] + w0[i]*C_k[80+i]

Division of labor, all ops wide (FD = 8 chunks * 160 = 1280) to
amortize fixed per-instruction costs:
  DVE: 37 taps/seq as one wide mult each (in1 = per-(frame,chunk) h
       broadcast via stride-0 inner AP; 1x mode) -> product planes
  ACT: 13 taps/seq as 8 chunk-wise Copy-with-scale products -> planes
  PE : accumulates every plane into PSUM via identity-stationary
       matmuls (3 windows 480|480|320; fp32 accumulation; ldw-opt
       dedups the identity LDWEIGHTS); also applies the +1 partition
       shift with a constant superdiagonal matrix (Vs = SH.T @ V),
       replacing a slow strided SBUF->SBUF DMA
  DVE tail: V = psum*rr; y = V_hi + Vs_psum (fp32 out)
GpSimd idle by design: its tensor ops halve concurrent DVE throughput
(shared SBUF port).

Hard-won rules encoded here:
  - PE must never write PSUM (any bank) while DVE reads PSUM: PE is
    gated on v_s / y_s around the V and blend phases.
  - Completion semaphores ride on the last real engine instruction,
    never on a trailing nop (fires before queued PSUM writes land).
  - Per-DMA semaphores (completion skew across SDMA engines makes
    shared counters unsound).
  - Input is host-prepped into the unfolded [126, chunk, 212] layout
    (contiguous DMA; the strided gather cost ~12us of head latency),
    output staged [125, 1280] and de-interleaved on host.

Sharding: pure data parallel across batch, 2 sequences per core.
"""

import sys

for p in ("/opt/trn_rl_repo", "/root/.axon_site/_ro/trn_rl_repo"):
    if p not in sys.path:
        sys.path.append(p)

import numpy as np
import concourse.bass as bass
import concourse.mybir as mybir
from concourse.ap import AP
from concourse.bass_utils import run_bass_kernel_spmd


def _enable_ldw_opt():
    """Dedup identical LDWEIGHTS (walrus --enable-ldw-opt): saves ~40us of
    redundant identity reloads on the PE. Idempotent monkeypatch."""
    from concourse import bass_utils as _bu

    if getattr(_bu.run_command, "_ldw_opt_patched", False):
        return
    _orig = _bu.run_command

    def _patched(cmd, **kw):
        cmd = [
            "--enable-ldw-opt=true" if c == "--enable-ldw-opt=false" else c
            for c in cmd
        ]
        return _orig(cmd, **kw)

    _patched._ldw_opt_patched = True
    _bu.run_command = _patched


_enable_ldw_opt()

B, T = 16, 80000
P, D = 80, 50
N = T // P
NCORES = 8
S = B // NCORES
FO = 125
NC = 8
CS = 212
PAD = D - 1 + P  # 129
GF = NC * 160  # 1280
XF = NC * CS  # 1696
WIN = [(0, 480), (480, 480), (960, 320)]

F16 = mybir.dt.float16
FP32 = mybir.dt.float32

DVE_TAPS = list(range(0, 36))
GPS_TAPS = []
ACT_TAPS = list(range(36, 50))

_nc_cache = {}


def plane_schedule():
    """PE consume order. D-planes arrive every ~1.5us, A-planes every
    ~4.04us; A-ready is biased +2.5us so the fast consumer doesn't
    head-of-line block on the slow producer while D-planes queue."""
    ev = [("D", k, 1500.0 * (k + 1)) for k in range(len(DVE_TAPS))]
    ev += [("A", k, 4040.0 * (k + 1) + 6500.0) for k in range(len(ACT_TAPS))]
    ev.sort(key=lambda e: e[2])
    return [(kind, k) for kind, k, _ in ev]


def build_nc():
    if "nc" in _nc_cache:
        return _nc_cache["nc"]
    nc = bass.Bass()
    xa_ext = nc.declare_dram_parameter("xae", [S, 126, XF], F16, isOutput=False)
    hg_ext = nc.declare_dram_parameter("hg", [126, D * 16], F16, isOutput=False)
    hg32_ext = nc.declare_dram_parameter("hg32", [126, D * 16], FP32, isOutput=False)
    rr_ext = nc.declare_dram_parameter("rr", [128, 160], F16, isOutput=False)
    eye_ext = nc.declare_dram_parameter("eye", [126, 126], F16, isOutput=False)
    shm_ext = nc.declare_dram_parameter("shm", [126, 126], F16, isOutput=False)
    out_ext = nc.declare_dram_parameter("out", [S, FO, GF // 2], FP32, isOutput=True)

    from contextlib import ExitStack

    with ExitStack() as _ctx:
        ec = _ctx.enter_context
        xa0 = ec(nc.sbuf_tensor([126, XF], F16))
        xa1 = ec(nc.sbuf_tensor([126, XF], F16))
        hgx = ec(nc.sbuf_tensor([126, D * 16], F16))
        hgx32 = ec(nc.sbuf_tensor([126, D * 16], FP32))
        rrt = ec(nc.sbuf_tensor([128, 160], F16))
        eye = ec(nc.sbuf_tensor([126, 126], F16))
        shm = ec(nc.sbuf_tensor([126, 126], F16))
        dpb = [ec(nc.sbuf_tensor(f"dpl{i}", [126, GF], F16)) for i in range(12)]
        gpb = [ec(nc.sbuf_tensor(f"gpl{i}", [126, GF], F16)) for i in range(2)]
        apb = [ec(nc.sbuf_tensor(f"apl{i}", [126, GF], F16)) for i in range(6)]
        vt0 = ec(nc.sbuf_tensor([126, GF], F16))
        vt1 = ec(nc.sbuf_tensor([126, GF], F16))
        yy0 = ec(nc.sbuf_tensor([125, GF // 2], FP32))
        yy1 = ec(nc.sbuf_tensor([125, GF // 2], FP32))
        ps0_0 = ec(nc.psum_tensor("ps0_0", [126, 480], FP32))
        ps0_1 = ec(nc.psum_tensor("ps0_1", [126, 480], FP32))
        ps0_2 = ec(nc.psum_tensor("ps0_2", [126, 320], FP32))
        ps1_0 = ec(nc.psum_tensor("ps1_0", [126, 480], FP32))
        ps1_1 = ec(nc.psum_tensor("ps1_1", [126, 480], FP32))
        ps1_2 = ec(nc.psum_tensor("ps1_2", [126, 320], FP32))
        xin0 = ec(nc.semaphore("xin0"))
        xin0b = ec(nc.semaphore("xin0b"))
        xin1 = ec(nc.semaphore("xin1"))
        xin1b = ec(nc.semaphore("xin1b"))
        o_s2 = ec(nc.semaphore("o_s2"))
        cin = ec(nc.semaphore("cin"))
        hgin = ec(nc.semaphore("hgin"))
        pein = ec(nc.semaphore("pein"))
        dp_s = ec(nc.semaphore("dp_s"))
        dc_s = ec(nc.semaphore("dc_s"))
        ap_s = ec(nc.semaphore("ap_s"))
        ac_s = ec(nc.semaphore("ac_s"))
        gp_s = ec(nc.semaphore("gp_s"))
        gc_s = ec(nc.semaphore("gc_s"))
        pe_d = ec(nc.semaphore("pe_d"))
        v_s = ec(nc.semaphore("v_s"))
        vsr = ec(nc.semaphore("vsr"))
        y_s = ec(nc.semaphore("y_s"))
        o_s = ec(nc.semaphore("o_s"))
        block = ec(nc.Block())

        xa = [xa0, xa1]
        xin = [xin0, xin1]
        vt = [vt0, vt1]
        yy = [yy0, yy1]
        psall = [[ps0_0, ps0_1, ps0_2], [ps1_0, ps1_1, ps1_2]]

        def xa3(s, j):
            return AP(
                tensor=xa[s][:].tensor,
                offset=(D - 1 - j),
                ap=[[XF, 126], [CS, NC], [1, 160]],
            )

        def xa2(s, j, c):
            return xa[s][:, c * CS + D - 1 - j : c * CS + D - 1 - j + 160]

        def hb(s, j):
            return AP(
                tensor=hgx[:].tensor,
                offset=j * 16 + s * NC,
                ap=[[D * 16, 126], [1, NC], [0, 160]],
            )

        def rrb(nw):
            return AP(
                tensor=rrt[:].tensor, offset=0, ap=[[160, 126], [0, nw], [1, 160]]
            )

        @block.sync
        def _(sync):
            sync.dma_start(xa[0][0:63, :], xa_ext[0, 0:63, :]).then_inc(xin[0], 16)
            sync.dma_start(hgx[:], hg_ext[:]).then_inc(hgin, 16)
            sync.dma_start(rrt[:], rr_ext[:]).then_inc(cin, 16)
            sync.dma_start(hgx32[:], hg32_ext[:]).then_inc(cin, 16)
            sync.dma_start(eye[:], eye_ext[:]).then_inc(pein, 16)
            sync.dma_start(shm[:], shm_ext[:]).then_inc(pein, 16)
            sync.dma_start(xa[1][0:63, :], xa_ext[1, 0:63, :]).then_inc(xin[1], 16)
            sync.dma_start(xa[1][63:126, :], xa_ext[1, 63:126, :]).then_inc(
                xin1b, 16
            )
            sync.wait_ge(y_s, 1)
            sync.dma_start(out_ext[0, :, :], yy[0][:]).then_inc(o_s, 16)
            sync.wait_ge(y_s, 2)
            sync.dma_start(out_ext[1, :, 0:320], yy[1][:, 0:320]).then_inc(o_s, 16)
            sync.wait_ge(o_s, 32)
            sync.wait_ge(o_s2, 16)

        def emit_vmult(vector, s):
            if s == 0:
                vector.wait_ge(cin, 32)
            vector.wait_ge(pe_d, s + 1)
            for w, (o, ln) in enumerate(WIN):
                inst = vector.tensor_tensor(
                    out=vt[s][:, o : o + ln],
                    in0=psall[s][w][:],
                    in1=rrb(ln // 160),
                    op=mybir.AluOpType.mult,
                )
            inst.then_inc(v_s, 1)

        def emit_yblend(vector, s):
            # y[q, c, i] = V[q, c, 80+i] + Vs[q, c, i]; Vs lives in psum
            vector.wait_ge(vsr, s + 1)
            for w, (o, ln) in enumerate(WIN):
                nw = ln // 160
                inst = vector.tensor_tensor(
                    out=AP(
                        tensor=yy[s][:].tensor,
                        offset=o // 2,
                        ap=[[GF // 2, 125], [P, nw], [1, P]],
                    ),
                    in0=AP(
                        tensor=vt[s][:].tensor,
                        offset=o + P,
                        ap=[[GF, 125], [160, nw], [1, P]],
                    ),
                    in1=AP(
                        tensor=psall[s][w][:].tensor,
                        offset=0,
                        ap=[[ln, 125], [160, nw], [1, P]],
                    ),
                    op=mybir.AluOpType.add,
                )
            inst.then_inc(y_s, 1)

        @block.vector
        def _(vector):
            vector.wait_ge(hgin, 16)
            nd = len(DVE_TAPS)
            xinb = [xin0b, xin1b]
            for s in range(S):
                vector.wait_ge(xin[s], 16)
                vector.wait_ge(xinb[s], 16)
                for k, j in enumerate(DVE_TAPS):
                    g = s * nd + k + 1
                    if g > 12:
                        vector.wait_ge(dc_s, (g - 12 + 3) // 4)
                    vector.tensor_tensor(
                        out=dpb[(g - 1) % 12][:],
                        in0=xa3(s, j),
                        in1=hb(s, j),
                        op=mybir.AluOpType.mult,
                    ).then_inc(dp_s, 1)
                    if s == 1 and k == 1:
                        emit_vmult(vector, 0)
                    if s == 1 and k == 5:
                        emit_yblend(vector, 0)
            emit_vmult(vector, 1)
            emit_yblend(vector, 1)

        @block.tensor
        def _(tensor):
            tensor.wait_ge(pein, 32)
            nd, na = len(DVE_TAPS), len(ACT_TAPS)
            sched = plane_schedule()
            nplanes = nd + na + len(GPS_TAPS)
            for s in range(S):
                if s > 0:
                    # PE may not touch PSUM while DVE reads it (V/blend of
                    # the previous sequence)
                    tensor.wait_ge(y_s, s)
                done = 0
                for kind, k in sched:
                    if kind == "D":
                        g = s * nd + k + 1
                        tensor.wait_ge(dp_s, g)
                        src = dpb[(g - 1) % 12]
                    elif kind == "G":
                        g = s * len(GPS_TAPS) + k + 1
                        tensor.wait_ge(gp_s, g)
                        src = gpb[(g - 1) % 2]
                    else:
                        g = s * na + k + 1
                        tensor.wait_ge(ap_s, g)
                        src = apb[(g - 1) % 6]
                    first = done == 0
                    done += 1
                    last = done == nplanes
                    for w, (o, ln) in enumerate(WIN):
                        inst = tensor.matmul(
                            psall[s][w][:],
                            eye[:],
                            src[:, o : o + ln],
                            start=first,
                            stop=last,
                            skip_group_check=True,
                        )
                    if done == nplanes:
                        inst.then_inc(pe_d, 1)
                    elif kind == "D" and g % 4 == 0:
                        inst.then_inc(dc_s, 1)
                    elif kind == "G":
                        inst.then_inc(gc_s, 1)
                    elif kind == "A" and g % 2 == 0:
                        inst.then_inc(ac_s, 1)
                # Vs = SH.T @ V (the +1 partition shift), reusing this
                # sequence's psum banks after the V-mult consumed them
                tensor.wait_ge(v_s, s + 1)
                for w, (o, ln) in enumerate(WIN):
                    inst = tensor.matmul(
                        psall[s][w][:],
                        shm[:],
                        vt[s][:, o : o + ln],
                        start=True,
                        stop=True,
                        skip_group_check=True,
                    )
                inst.then_inc(vsr, 1)

        @block.scalar
        def _(scalar):
            scalar.dma_start(xa[0][63:126, :], xa_ext[0, 63:126, :]).then_inc(
                xin0b, 16
            )
            scalar.wait_ge(cin, 32)
            xinb = [xin0b, xin1b]
            na = len(ACT_TAPS)
            for s in range(S):
                scalar.wait_ge(xinb[s], 16)
                scalar.wait_ge(xin[s], 16)
                for k, j in enumerate(ACT_TAPS):
                    g = s * na + k + 1
                    if g > 6:
                        scalar.wait_ge(ac_s, (g - 6 + 1) // 2)
                    for c in range(NC):
                        inst = scalar.activation(
                            apb[(g - 1) % 6][:, c * 160 : (c + 1) * 160],
                            xa2(s, j, c),
                            mybir.ActivationFunctionType.Copy,
                            scale=hgx32[
                                :, j * 16 + s * NC + c : j * 16 + s * NC + c + 1
                            ],
                        )
                    inst.then_inc(ap_s, 1)
            scalar.wait_ge(y_s, 2)
            scalar.dma_start(out_ext[1, :, 320:640], yy[1][:, 320:640]).then_inc(
                o_s2, 16
            )

    _nc_cache["nc"] = nc
    return nc


def _prep_core_inputs(x, h):
    x = np.ascontiguousarray(x, dtype=np.float32)
    h = np.ascontiguousarray(h, dtype=np.float32)
    TPX = N * P + CS + 4
    xp = np.zeros((B, TPX), np.float16)
    xp[:, PAD : PAD + T] = x.astype(np.float16)
    # host im2col: xae[b, q, c*CS + u] = xp[b, (c*FO + q)*P + u]
    idx_q = (np.arange(126)[:, None, None] * P + np.arange(NC)[None, :, None] * FO * P
             + np.arange(CS)[None, None, :])  # [126, NC, CS]
    xae_all = xp[:, idx_q.reshape(-1)].reshape(B, 126, NC * CS)
    hpad = np.concatenate([h, h[:, -1:, :]], axis=1)
    w1 = (np.arange(P, dtype=np.float32) / P).astype(np.float16)
    w0 = (1.0 - np.arange(P, dtype=np.float32) / P).astype(np.float16)
    rr = np.ascontiguousarray(
        np.broadcast_to(np.concatenate([w1, w0])[None, :], (128, 160))
    )
    eye = np.eye(126, dtype=np.float16)
    shm = np.zeros((126, 126), np.float16)
    shm[np.arange(1, 126), np.arange(125)] = 1.0  # SH[k', m] = 1 iff k' == m+1
    in_maps = []
    for core in range(NCORES):
        sl = slice(core * S, (core + 1) * S)
        hc = hpad[sl]
        hg32 = np.zeros((126, D * 16), np.float32)
        for s in range(S):
            for c in range(NC):
                blk = hc[s, c * FO : c * FO + 126, :]
                hg32[:, s * NC + c :: 16] = blk
        in_maps.append(
            {
                "xae": np.ascontiguousarray(xae_all[sl]),
                "hg": hg32.astype(np.float16),
                "hg32": hg32,
                "rr": rr,
                "eye": eye,
                "shm": shm,
            }
        )
    return in_maps


def _unstage(res):
    outs = []
    for c in range(NCORES):
        o = res.results[c]["out"]  # [S, 125, 8*80]
        o = o.reshape(S, FO, NC, P).transpose(0, 2, 1, 3).reshape(S, T)
        outs.append(o)
    return np.ascontiguousarray(np.concatenate(outs, axis=0), dtype=np.float32)


def kernel(x, h, **kw):
    nc = build_nc()
    in_maps = _prep_core_inputs(x, h)
    res = run_bass_kernel_spmd(nc, in_maps, core_ids=list(range(NCORES)), **kw)
    return _unstage(res)


def kernel_traced(x, h, **kw):
    nc = build_nc()
    in_maps = _prep_core_inputs(x, h)
    res = run_bass_kernel_spmd(
        nc, in_maps, core_ids=list(range(NCORES)), trace=True, **kw
    )
    return _unstage(res), res
